# revision 14
# baseline (speedup 1.0000x reference)
"""Trainium2 Bass kernel for nn_DNFLayer (fuzzy DNF layer), v2.2.

Strategy
--------
Data-parallel over batch B=32 across 8 cores (4 batches/core). Per core the
(i, j) permutation grid is padded to the full 32x32 grid (diagonal masked via
the OR-kernel broadcast), 4096 rows as 32 row-tiles of 128 partitions with
row layout p = j*4 + i4, i = t*4 + i4, j = p//4 (t = tile index within b).
This layout makes the final per-(b,i) probsum a short partition tree plus one
32x32 transpose, and the per-b probsum a Ln/partition-all-reduce/Exp.

conj = F0(b) * FU1(b,i) * FU2(b,j) * FB1(b,i,j) * FB2(b,j,i), each factor a
product of gamma-form terms (gamma*x + 1); beta products fold into the
OR-kernel broadcast (exact, via Ln / partition all-reduce / Exp).

Engine split: DVE runs the big eval multiplies, first tree levels and most
reductions; Scalar runs +1 biases (b0-b2) and PSUM evacuations; Pool runs two
of the t2 tree levels, the partition all-reduces and the binary merge; PE does
all broadcasts in bf16 plus phase-D selector matmuls.  Activation tables are
preloaded in first-use order (Exp, Ln early; Copy loads while PE broadcasts
run) and DMA issue work is spread across the idle Sync/PE/Pool queues.
"""

import numpy as np
import ml_dtypes

BF = ml_dtypes.bfloat16
B, N, P0, P1, P2, R, D = 32, 32, 16, 32, 16, 3, 8
RD = R * D              # 24
NCORE = 8
BL = B // NCORE         # 4 batches per core

_CACHE = {}


def _build():
    import concourse.tile as tile
    from concourse import mybir, bacc, bass_isa

    F32 = mybir.dt.float32
    B16 = mybir.dt.bfloat16
    MUL = mybir.AluOpType.mult
    ADD = mybir.AluOpType.add
    SUB = mybir.AluOpType.subtract
    AF = mybir.ActivationFunctionType
    AX = mybir.AxisListType.X

    nc = bacc.Bacc("TRN2", target_bir_lowering=False, debug=False,
                   num_devices=NCORE)

    # ---- parameters ----
    x_all_in = nc.declare_dram_parameter("x_all", [128, 1024], B16, isOutput=False)
    xu_in = nc.declare_dram_parameter("xu", [128, 80], B16, isOutput=False)
    akt_in = nc.declare_dram_parameter("akt", [112, 72], F32, isOutput=False)
    ork_in = nc.declare_dram_parameter("ork", [1, 24], F32, isOutput=False)
    selPJ_in = nc.declare_dram_parameter("selPJ", [32, 256], B16, isOutput=False)
    masks_in = nc.declare_dram_parameter("masks", [128, 384], B16, isOutput=False)
    om_in = nc.declare_dram_parameter("om_all", [128, 72], F32, isOutput=False)

    out_binm = nc.declare_dram_parameter("out_binm", [128, 32], F32, isOutput=True)
    out_unm = nc.declare_dram_parameter("out_unm", [4, 32], F32, isOutput=True)
    out_nullm = nc.declare_dram_parameter("out_nullm", [1, 4], F32, isOutput=True)

    with tile.TileContext(nc) as tc:
        with tc.tile_pool(name="cb", bufs=1) as cb, \
             tc.tile_pool(name="wk", bufs=1) as wk, \
             tc.tile_pool(name="ps", bufs=4, space="PSUM") as ps, \
             tc.tile_pool(name="ps2", bufs=2, space="PSUM") as ps2:

            # ---------- input DMAs: early tensors on Sync, late on Scalar ---
            akt = cb.tile([112, 72], F32)
            nc.sync.dma_start(akt[:], akt_in[:])
            okt = cb.tile([1, 24], F32)
            nc.sync.dma_start(okt[:], ork_in[:])
            x_all = cb.tile([128, 1024], B16)
            for h in range(4):
                nc.sync.dma_start(x_all[:, h * 256:(h + 1) * 256],
                                  x_all_in[:, h * 256:(h + 1) * 256])
            xu = cb.tile([128, 80], B16)
            nc.sync.dma_start(xu[:], xu_in[:])
            selPJ = cb.tile([32, 256], B16)
            nc.scalar.dma_start(selPJ[:], selPJ_in[:])
            masks = cb.tile([128, 384], B16)
            nc.scalar.dma_start(masks[:], masks_in[:])
            om_all = cb.tile([128, 72], F32)
            nc.scalar.dma_start(om_all[:], om_in[:])
            nq = cb.tile([32, 32], F32)
            nc.vector.memset(nq[:], 1.0)
            ones128 = cb.tile([128, 1], F32)
            nc.vector.memset(ones128[:], 1.0)
            ones1b = cb.tile([1, 128], B16)
            nc.vector.memset(ones1b[:], 1.0)
            gamPb = cb.tile([128, 32], B16)
            nc.vector.memset(gamPb[:], 1.0)
            selP = selPJ[:, 0:128]
            selJ = selPJ[:, 128:256]
            maskm = masks[:, 0:192]
            maskc2 = masks[:, 192:384]
            omb = om_all[:, 0:32]
            omuT = om_all[0:4, 32:64]
            omn = om_all[0:1, 64:68]
            selU = om_all[:, 68:72]

            # ---------- phase A ----------
            e = wk.tile([112, 72], F32)
            nc.scalar.activation(e[:], akt[:], AF.Exp)
            eok = wk.tile([1, 24], F32)
            nc.scalar.activation(eok[:], okt[:], AF.Exp, scale=-1.0)
            e3 = e[:].rearrange("p (r m) -> p r m", m=3)
            bsum = wk.tile([112, 24], F32)
            nc.vector.tensor_tensor(bsum[:], e3[:, :, 1], e3[:, :, 2], op=ADD)
            stot = wk.tile([112, 24], F32)
            nc.vector.tensor_tensor(stot[:], e3[:, :, 0], bsum[:], op=ADD)
            gam = wk.tile([112, 24], F32)
            nc.vector.tensor_tensor(gam[:], e3[:, :, 0], e3[:, :, 1], op=SUB)
            rbs = wk.tile([112, 24], F32)
            nc.vector.reciprocal(rbs[:], bsum[:])
            nc.vector.tensor_tensor(gam[:], gam[:], rbs[:], op=MUL)

            # transpose gamma (bf16) first — unblocks the gB broadcast chain
            nc.vector.tensor_copy(gamPb[0:112, 0:24], gam[:])
            gamTb = wk.tile([32, 128], B16)
            for blk in range(4):
                nc.vector.transpose(gamTb[0:32, blk * 32:(blk + 1) * 32],
                                    gamPb[blk * 32:(blk + 1) * 32, 0:32])
            g_flat = wk.tile([1, 2688], B16)
            nc.sync.dma_start(g_flat[:].rearrange("p (r k) -> p r k", r=24),
                              gamTb[0:24, 0:112])
            gfv = g_flat[:].rearrange("p (r k) -> p r k", r=24)

            lnb = wk.tile([112, 24], F32)
            nc.scalar.activation(lnb[:], bsum[:], AF.Ln)
            lns = wk.tile([112, 24], F32)
            nc.scalar.activation(lns[:], stot[:], AF.Ln)
            lnd = wk.tile([112, 24], F32)
            nc.gpsimd.tensor_tensor(lnd[:], lnb[:], lns[:], op=SUB)
            psb = ps2.tile([1, 24], F32, tag="po")
            nc.tensor.matmul(psb[:], ones128[0:112, :], lnd[:], start=True, stop=True)
            bA = wk.tile([1, 24], F32)
            nc.scalar.activation(bA[:], psb[:], AF.Exp)
            sd = wk.tile([1, 24], F32)
            nc.vector.tensor_scalar(sd[:], eok[:], 1.0, None, op0=ADD)
            sig = wk.tile([1, 24], F32)
            nc.vector.reciprocal(sig[:], sd[:])
            nc.vector.tensor_tensor(sig[:], sig[:], bA[:], op=MUL)
            sigb = wk.tile([1, 24], B16)
            nc.vector.tensor_copy(sigb[:], sig[:])

            # ---------- phase B: bf16 PE broadcasts ----------
            gB = wk.tile([128, 768], B16)     # (rd, c32), k = 80 + c
            gU = wk.tile([128, 1536], B16)    # (s2, rd, c32), k = 16 + s*32 + c
            gN = wk.tile([128, 384], B16)     # (rd, c16), k = c
            # gB first; evacuate on DVE (unblocks em0 without the Copy table)
            gb_ps = []
            for rhs, off in ((gfv[:, 0:16, 80:112], 0),
                             (gfv[:, 16:24, 80:112], 512)):
                n = rhs.free_size()
                pb = ps.tile([128, n], F32, tag="mm")
                nc.tensor.matmul(pb[:], ones1b[:], rhs, start=True, stop=True)
                gb_ps.append((pb, off, n))
            for pb, off, n in gb_ps:
                nc.vector.tensor_copy(gB[:, off:off + n], pb[:])
            # gU / gN; evacuate on Scalar (loads the Copy table meanwhile)
            for rhs, dst, off in ((gfv[:, 0:16, 16:48], gU, 0),
                                  (gfv[:, 16:24, 16:48], gU, 512),
                                  (gfv[:, 0:16, 48:80], gU, 768),
                                  (gfv[:, 16:24, 48:80], gU, 1280),
                                  (gfv[:, 0:24, 0:16], gN, 0)):
                n = rhs.free_size()
                pb = ps.tile([128, n], F32, tag="mm")
                nc.tensor.matmul(pb[:], ones1b[:], rhs, start=True, stop=True)
                nc.scalar.activation(dst[:, off:off + n], pb[:], AF.Copy)

            em = wk.tile([128, 4 * 6144], B16)
            t1 = wk.tile([128, 4 * 3072], B16)
            t2 = wk.tile([128, 4 * 1536], B16)
            t3 = wk.tile([128, 4 * 768], B16)
            t4 = wk.tile([128, 4 * 384], B16)
            cj = wk.tile([128, 768], B16)     # (b, t, rd)

            def em_eval(b):
                nc.vector.tensor_tensor(
                    em[:, b * 6144:(b + 1) * 6144]
                        .rearrange("p (t r c) -> p t r c", t=8, r=24),
                    x_all[:, b * 256:(b + 1) * 256]
                        .rearrange("p (t c) -> p t c", t=8)
                        .unsqueeze(2).broadcast_to((128, 8, 24, 32)),
                    gB[:].rearrange("p (r c) -> p r c", r=24)
                        .unsqueeze(1).broadcast_to((128, 8, 24, 32)), op=MUL)

            def em_bias(b, eng):
                sl = em[:, b * 6144:(b + 1) * 6144]
                if eng == "S":
                    nc.scalar.activation(sl, sl, AF.Copy, bias=1.0)
                else:
                    nc.vector.tensor_scalar(sl, sl, 1.0, None, op0=ADD)

            def t1_eval(b):
                src = em[:, b * 6144:(b + 1) * 6144].rearrange(
                    "p (g c) -> p g c", c=32)
                nc.vector.tensor_tensor(
                    t1[:, b * 3072:(b + 1) * 3072]
                        .rearrange("p (g c) -> p g c", c=16),
                    src[:, :, 0:16], src[:, :, 16:32], op=MUL)

            def t2_eval(b):
                src = t1[:, b * 3072:(b + 1) * 3072].rearrange(
                    "p (g c) -> p g c", c=16)
                dst = t2[:, b * 1536:(b + 1) * 1536].rearrange(
                    "p (g c) -> p g c", c=8)
                nc.vector.tensor_tensor(dst, src[:, :, 0:8],
                                        src[:, :, 8:16], op=MUL)

            def t34cj(b):
                src = t2[:, b * 1536:(b + 1) * 1536].rearrange(
                    "p (g c) -> p g c", c=8)
                d3 = t3[:, b * 768:(b + 1) * 768].rearrange(
                    "p (g c) -> p g c", c=4)
                nc.vector.tensor_tensor(d3, src[:, :, 0:4], src[:, :, 4:8],
                                        op=MUL)
                d4 = t4[:, b * 384:(b + 1) * 384].rearrange(
                    "p (g c) -> p g c", c=2)
                nc.vector.tensor_tensor(d4, d3[:, :, 0:2], d3[:, :, 2:4],
                                        op=MUL)
                nc.vector.tensor_tensor(
                    cj[:, b * 192:(b + 1) * 192].unsqueeze(2),
                    d4[:, :, 0:1], d4[:, :, 1:2], op=MUL)

            # ---------- interleaved main pipeline ----------
            em_eval(0)
            em_bias(0, "S")

            # phase C (fits while Scalar runs bias0)
            emU = wk.tile([128, 1536], B16)   # (s2, rd24, c32)
            for s in range(2):
                nc.vector.tensor_tensor(
                    emU[:, s * 768:(s + 1) * 768]
                        .rearrange("p (r c) -> p r c", c=32),
                    xu[:, s * 32:(s + 1) * 32].unsqueeze(1)
                        .broadcast_to((128, 24, 32)),
                    gU[:, s * 768:(s + 1) * 768]
                        .rearrange("p (r c) -> p r c", c=32), op=MUL)
            nc.vector.tensor_scalar(emU[:], emU[:], 1.0, None, op0=ADD)
            emN = wk.tile([128, 384], B16)    # (rd24, c16)
            nc.vector.tensor_tensor(
                emN[:].rearrange("p (r c) -> p r c", r=24),
                xu[:, 64:80].unsqueeze(1).broadcast_to((128, 24, 16)),
                gN[:].rearrange("p (r c) -> p r c", r=24), op=MUL)
            nc.vector.tensor_scalar(emN[:], emN[:], 1.0, None, op0=ADD)

            cur = emU[:].rearrange("p (g c) -> p g c", c=32)
            for w in (16, 8, 4, 2):
                nxt = wk.tile([128, 48 * w], B16, tag=f"ut{w}")
                nc.vector.tensor_tensor(
                    nxt[:].rearrange("p (g c) -> p g c", c=w),
                    cur[:, :, 0:w], cur[:, :, w:2 * w], op=MUL)
                cur = nxt[:].rearrange("p (g c) -> p g c", c=w)
            fu12 = wk.tile([128, 48], B16)
            nc.vector.tensor_tensor(fu12[:].unsqueeze(2), cur[:, :, 0:1],
                                    cur[:, :, 1:2], op=MUL)
            cur = emN[:].rearrange("p (g c) -> p g c", c=16)
            for w in (8, 4, 2):
                nxt = wk.tile([128, 24 * w], B16, tag=f"nt{w}")
                nc.vector.tensor_tensor(
                    nxt[:].rearrange("p (g c) -> p g c", c=w),
                    cur[:, :, 0:w], cur[:, :, w:2 * w], op=MUL)
                cur = nxt[:].rearrange("p (g c) -> p g c", c=w)
            f0g = wk.tile([128, 24], B16)
            nc.vector.tensor_tensor(f0g[:].unsqueeze(2), cur[:, :, 0:1],
                                    cur[:, :, 1:2], op=MUL)
            fu2f0 = wk.tile([128, 24], B16)
            nc.vector.tensor_tensor(fu2f0[:], fu12[:, 24:48], f0g[:], op=MUL)

            em_eval(1)
            em_bias(1, "S")

            # phase D small builds
            rhsAll = wk.tile([32, 768], B16)
            for b in range(BL):
                nc.vector.tensor_tensor(
                    rhsAll[:, b * 192:(b + 1) * 192]
                        .rearrange("p (t r) -> p t r", t=8),
                    fu12[b * 32:(b + 1) * 32, 0:24].unsqueeze(1)
                        .broadcast_to((32, 8, 24)),
                    maskm[b * 32:(b + 1) * 32]
                        .rearrange("p (t r) -> p t r", t=8), op=MUL)
            rhs2 = wk.tile([32, 96], B16)
            for b in range(BL):
                nc.vector.tensor_copy(rhs2[:, b * 24:(b + 1) * 24],
                                      fu2f0[b * 32:(b + 1) * 32, :])

            FU1B = wk.tile([128, 768], B16)   # (b, t, rd)
            for b in range(BL):
                pf = ps2.tile([128, 192], F32, tag="pf")
                nc.tensor.matmul(pf[:], selP,
                                 rhsAll[:, b * 192:(b + 1) * 192],
                                 start=True, stop=True)
                nc.scalar.activation(FU1B[:, b * 192:(b + 1) * 192], pf[:],
                                     AF.Copy)
            FU2F0B = wk.tile([128, 96], B16)  # (b, rd)
            pj = ps2.tile([128, 96], F32, tag="pf")
            nc.tensor.matmul(pj[:], selJ, rhs2[:], start=True, stop=True)
            nc.scalar.activation(FU2F0B[:], pj[:], AF.Copy)
            psO = ps2.tile([128, 24], F32, tag="po")
            nc.tensor.matmul(psO[:], ones1b[:], sigb[:], start=True, stop=True)

            t1_eval(0)
            em_eval(2)
            em_bias(2, "S")
            t2_eval(0)
            t34cj(0)
            t1_eval(1)
            em_eval(3)
            em_bias(3, "V")
            t2_eval(1)
            t34cj(1)
            t1_eval(2)
            t2_eval(2)
            t34cj(2)
            t1_eval(3)
            t2_eval(3)
            t34cj(3)

            # okmB / PFOK (needed just before post-cj)
            okmB = wk.tile([128, 192], B16)
            nc.vector.tensor_tensor(
                okmB[:].rearrange("p (t r) -> p t r", t=8),
                maskc2.rearrange("p (t r) -> p t r", t=8),
                psO[:].unsqueeze(1).broadcast_to((128, 8, 24)), op=MUL)
            PFOK = wk.tile([128, 768], B16)
            nc.vector.tensor_tensor(
                PFOK[:].rearrange("p (b t r) -> p b t r", b=4, t=8),
                FU1B[:].rearrange("p (b t r) -> p b t r", b=4, t=8),
                FU2F0B[:].rearrange("p (b r) -> p b r", b=4)
                    .unsqueeze(2).broadcast_to((128, 4, 8, 24)), op=MUL)
            nc.vector.tensor_tensor(
                PFOK[:].rearrange("p (b t r) -> p b t r", b=4, t=8),
                PFOK[:].rearrange("p (b t r) -> p b t r", b=4, t=8),
                okmB[:].rearrange("p (t r) -> p t r", t=8)
                    .unsqueeze(1).broadcast_to((128, 4, 8, 24)), op=MUL)

            # prefetch Ln/Exp tables for phase F while Scalar is idle; the
            # input dep on cj keeps these after every Copy-table use
            dum = wk.tile([1, 16], F32)
            nc.scalar.activation(dum[:], cj[0:1, 0:16], AF.Ln)
            nc.scalar.activation(dum[:], cj[0:1, 0:16], AF.Exp)

            # ---------- post-cj (batched over all b) ----------
            cjb = wk.tile([128, 768], B16)
            nc.vector.tensor_tensor(cjb[:], cj[:], PFOK[:], op=MUL)
            gA = wk.tile([128, 768], B16)
            nc.vector.tensor_scalar(gA[:], cjb[:], -1.0, 1.0, op0=MUL, op1=ADD)
            d1 = wk.tile([128, 384], B16)
            gv = gA[:].rearrange("p (g d) -> p g d", d=8)
            nc.vector.tensor_tensor(d1[:].rearrange("p (g d) -> p g d", d=4),
                                    gv[:, :, 0:4], gv[:, :, 4:8], op=MUL)
            d2 = wk.tile([128, 192], B16)
            dv = d1[:].rearrange("p (g d) -> p g d", d=4)
            nc.vector.tensor_tensor(d2[:].rearrange("p (g d) -> p g d", d=2),
                                    dv[:, :, 0:2], dv[:, :, 2:4], op=MUL)
            # pdF[p, (r, b, t)] fp32;  d2 is ((b,t), r, 2)
            pdF = wk.tile([128, 96], F32)
            d2v = d2[:].rearrange("p (g r d) -> p g r d", r=3, d=2)
            nc.vector.tensor_tensor(
                pdF[:].rearrange("p (r g) -> p g r", g=32).unsqueeze(3),
                d2v[:, :, :, 0:1], d2v[:, :, :, 1:2], op=MUL)

            # ---------- phase F: merges ----------
            # binary: out = 1 - omb * pd_r2
            tb = wk.tile([128, 32], F32)
            nc.vector.tensor_tensor(tb[:], omb, pdF[:, 64:96], op=MUL)
            nc.vector.tensor_scalar(tb[:], tb[:], -1.0, 1.0, op0=MUL, op1=ADD)
            nc.sync.dma_start(out_binm[:], tb[:])

            # log-space partition reductions: Ln -> PE matmul -> Exp
            lq = wk.tile([128, 64], F32)
            nc.scalar.activation(lq[:], pdF[:, 0:64], AF.Ln)
            pU = ps2.tile([4, 32], F32, tag="pf")
            nc.tensor.matmul(pU[:], selU, lq[:, 32:64], start=True, stop=True)
            pN = ps2.tile([1, 32], F32, tag="po")
            nc.tensor.matmul(pN[:], ones128[:], lq[:, 0:32], start=True,
                             stop=True)
            exU = wk.tile([4, 32], F32)
            nc.scalar.activation(exU[:], pU[:], AF.Exp)
            exN = wk.tile([1, 32], F32)
            nc.scalar.activation(exN[:], pN[:], AF.Exp)

            # unary merge: out = 1 - omuT * U   [4, (b,t)]
            tu = wk.tile([4, 32], F32)
            nc.vector.tensor_tensor(tu[:], omuT, exU[:], op=MUL)
            nc.vector.tensor_scalar(tu[:], tu[:], -1.0, 1.0, op0=MUL, op1=ADD)
            nc.sync.dma_start(out_unm[:], tu[:])

            # nullary: product over t, then merge
            cur = exN[:].rearrange("p (b t) -> p b t", b=4)
            for w in (4, 2, 1):
                nxt = wk.tile([1, 4 * w], F32, tag=f"tt{w}")
                nxtv = nxt[:].rearrange("p (b t) -> p b t", b=4)
                nc.vector.tensor_tensor(nxtv, cur[:, :, 0:w], cur[:, :, w:2 * w],
                                        op=MUL)
                cur = nxtv
            tn = wk.tile([1, 4], F32)
            nc.vector.tensor_tensor(tn[:], omn,
                                    cur.rearrange("p b t -> p (b t)"), op=MUL)
            nc.vector.tensor_scalar(tn[:], tn[:], -1.0, 1.0, op0=MUL, op1=ADD)
            nc.sync.dma_start(out_nullm[:], tn[:])

    nc.compile()
    return nc


def _host_prep(nullary_preds, unary_preds, binary_preds, and_kernel, or_kernel):
    """Build per-core input maps (sharding + layout prep only)."""
    null_ = np.asarray(nullary_preds, np.float32)
    un = np.asarray(unary_preds, np.float32)
    bi = np.asarray(binary_preds, np.float32)
    ak = np.asarray(and_kernel, np.float32)

    I, J = np.meshgrid(np.arange(N), np.arange(N), indexing="ij")
    off = I != J
    Jm = J - (J > I)
    Im = I - (I > J)

    binP = np.zeros((B, N, N, P2), np.float32)
    binP[:, off] = bi[:, I[off], Jm[off]]
    binT = np.zeros((B, N, N, P2), np.float32)
    binT[:, off] = bi[:, J[off], Im[off]]
    binPT = np.concatenate([binP, binT], axis=-1)          # [B,32,32,32]

    # row layout: p = j*4 + i4, i = t*4 + i4, j = p//4
    p = np.arange(128)
    t = np.arange(8)
    ii = t[None, :] * 4 + (p[:, None] % 4)   # [128, 8]
    jj = p // 4                              # [128]

    xg = binPT.reshape(NCORE, BL, N, N, 32)
    x_all = np.stack([xg[:, b, ii, jj[:, None], :] for b in range(BL)],
                     axis=1)  # [NCORE, BL, 128, 8, 32]
    x_all = np.ascontiguousarray(x_all.transpose(0, 2, 1, 3, 4)
                                 ).reshape(NCORE, 128, 1024).astype(BF)

    xun = np.concatenate(
        [un, un, np.broadcast_to(null_[:, None, :], (B, N, P0))], axis=-1)
    xu = xun.reshape(NCORE, 128, 80).astype(BF)

    akT = np.ascontiguousarray(ak.transpose(2, 0, 1, 3)).reshape(112, 72)
    ork = np.asarray(or_kernel, np.float32).reshape(1, 24)

    k = np.arange(32)
    selP = (k[:, None] % 4 == p[None, :] % 4)
    selJ = (k[:, None] == (p[None, :] // 4))
    selPJ = np.concatenate([selP, selJ], axis=1).astype(BF)       # [32,256]
    maskm = np.tile(np.repeat((k[:, None] // 4 == t[None, :]), RD, axis=1),
                    (4, 1))                                       # [128,192]
    maskc2 = np.repeat((ii != jj[:, None]), RD, axis=1)           # [128,192]
    masks = np.concatenate([maskm, maskc2], axis=1).astype(BF)    # [128,384]

    # 1 - old values, packed into one [128, 40] fp32 tensor
    ob = binP[..., 15]                                   # [B,32,32] diag=0
    omb = np.stack([1.0 - np.stack([ob[c * BL + b][ii, jj[:, None]]
                                    for b in range(BL)], axis=1)
                    for c in range(NCORE)])              # [NCORE,128,BL,8]
    omb = omb.reshape(NCORE, 128, 32)
    omuT = (1.0 - un[:, :, 31].reshape(NCORE, BL, 8, 4)
            ).reshape(NCORE, 32, 4).transpose(0, 2, 1)   # [i4, (b,t)]
    omn = (1.0 - null_[:, 15]).reshape(NCORE, 1, 4)
    om_all = np.zeros((NCORE, 128, 72), np.float32)
    om_all[:, :, 0:32] = omb
    om_all[:, 0:4, 32:64] = omuT
    om_all[:, 0:1, 64:68] = omn
    om_all[:, :, 68:72] = (p[:, None] % 4 == np.arange(4)[None, :])

    in_maps = []
    for c in range(NCORE):
        in_maps.append({
            "x_all": x_all[c],
            "xu": xu[c],
            "akt": akT,
            "ork": ork,
            "selPJ": selPJ,
            "masks": masks,
            "om_all": om_all[c],
        })
    return in_maps


def _assemble(results, nullary_preds, unary_preds, binary_preds):
    null_ = np.asarray(nullary_preds, np.float32).copy()
    un = np.asarray(unary_preds, np.float32).copy()
    bi = np.asarray(binary_preds, np.float32).copy()

    p = np.arange(128)
    t = np.arange(8)
    ii = t[None, :] * 4 + (p[:, None] % 4)   # [128, 8]
    jj = p // 4                              # [128]
    I, J = np.meshgrid(np.arange(N), np.arange(N), indexing="ij")
    off = I != J
    Jm = J - (J > I)

    for c in range(NCORE):
        r = results[c]
        ob = r["out_binm"].reshape(128, BL, 8)
        full = np.zeros((BL, N, N), np.float32)
        for b in range(BL):
            full[b, ii, jj[:, None]] = ob[:, b, :]
        for b in range(BL):
            bg = c * BL + b
            bi[bg, I[off], Jm[off], 15] = full[b][off]
        ou = r["out_unm"].reshape(4, BL, 8)
        un[c * BL:(c + 1) * BL, :, 31] = ou.transpose(1, 2, 0).reshape(BL, N)
        null_[c * BL:(c + 1) * BL, 15] = r["out_nullm"].reshape(BL)

    return np.concatenate(
        [null_, un.reshape(B, -1), bi.reshape(B, -1)], axis=-1)


def kernel(nullary_preds, unary_preds, binary_preds, and_kernel, or_kernel):
    from concourse.bass_utils import run_bass_kernel_spmd

    if "nc" not in _CACHE:
        _CACHE["nc"] = _build()
    nc = _CACHE["nc"]

    in_maps = _host_prep(nullary_preds, unary_preds, binary_preds,
                         and_kernel, or_kernel)
    res = run_bass_kernel_spmd(nc, in_maps, list(range(NCORE)))
    return _assemble(res.results, nullary_preds, unary_preds, binary_preds)


if __name__ == "__main__":
    import reference as ref
    ins = {k: np.asarray(v) for k, v in ref.setup_inputs().items()}
    out = kernel(**ins)
    print("kernel out:", out.shape, out.dtype)
